# revision 15
# baseline (speedup 1.0000x reference)
"""Trainium2 Bass kernel for nn_Attention_47605417509124 (sparse_attention).

Reference computation (B=4, N=4096, C=256), per batch b:
    g_x     = x @ g_w.T + g_b
    theta_x = x @ theta_w.T + theta_b
    phi_x   = x @ phi_w.T + phi_b
    f       = phi_x @ theta_x.T / N          # no softmax
    y       = f @ g_x
    out     = y @ W_w.T + W_b + x

Sharding: 8 cores = 4 batches x 2 sequence halves. Each core computes the
full theta/g projections for its batch (redundantly with its pair core) and
the phi rows / score rows / output rows for its own half of the sequence.

Host-side (free) prep:
  - x[b].T passed rotated so each core's own rows sit at columns 0..2047
    (exact: stage C sums over all j, so a consistent j-permutation of
    theta/g cancels).
  - all tensors pre-arranged in SBUF-native [128, ...] layouts so every DMA
    is contiguous; weights transposed; 1/N folded into g; W_b folded into
    the residual.

All matmuls run in float32r (TF32-like reduced-precision fp32, full PE rate)
with fp32 PSUM accumulation.
"""

import numpy as np

import concourse.bass as bass
import concourse.mybir as mybir
import concourse.tile as tile
from concourse import bacc
from concourse.bass_utils import run_bass_kernel_spmd

B, N, C = 4, 4096, 256
NCORES = 8
HALF = N // 2  # sequence rows handled per core
P = 128
JT = N // P          # 32 j tiles
IT = HALF // P       # 16 i tiles

F32 = mybir.dt.float32
F32R = mybir.dt.float32r
AF = mybir.ActivationFunctionType

_CACHE = {}


def _build_module():
    nc = bacc.Bacc("TRN2", target_bir_lowering=False, debug=False,
                   num_devices=NCORES)

    # ---- external I/O (per-core shapes, SBUF-native layouts) ----
    xT_d = nc.dram_tensor("xT", [P, 2, N], F32R, kind="ExternalInput")
    thW_d = nc.dram_tensor("thW", [P, 2, C], F32R, kind="ExternalInput")
    phW_d = nc.dram_tensor("phW", [P, 2, C], F32R, kind="ExternalInput")
    gW_d = nc.dram_tensor("gW", [P, 2, C], F32R, kind="ExternalInput")
    WW_d = nc.dram_tensor("WW", [P, 2, C], F32R, kind="ExternalInput")
    thb_d = nc.dram_tensor("thb", [P, 2], F32, kind="ExternalInput")
    phb_d = nc.dram_tensor("phb", [P, 2], F32, kind="ExternalInput")
    gbb_d = nc.dram_tensor("gbb", [P, 2, C], F32, kind="ExternalInput")
    resid_d = nc.dram_tensor("resid", [P, IT, C], F32, kind="ExternalInput")
    out_d = nc.dram_tensor("out", [P, IT, C], F32, kind="ExternalOutput")

    with tile.TileContext(nc) as tc:
        with tc.tile_pool(name="big", bufs=1) as big, \
             tc.tile_pool(name="fT", bufs=3) as fTp, \
             tc.tile_pool(name="ps_work", bufs=4, space="PSUM") as psw, \
             tc.tile_pool(name="ps_acc", bufs=4, space="PSUM") as psa:

            # ---- SBUF residents ----
            xT_sb = big.tile([P, 2, N], F32R)       # rotated x[b].T  32KB/part
            thW_sb = big.tile([P, 2, C], F32R)
            phW_sb = big.tile([P, 2, C], F32R)
            gW_sb = big.tile([P, 2, C], F32R)
            WW_sb = big.tile([P, 2, C], F32R)
            thb_sb = big.tile([P, 2], F32)
            phb_sb = big.tile([P, 2], F32)
            gbb_sb = big.tile([P, 2, C], F32)
            thetaT_sb = big.tile([P, 2, N], F32R)   # theta_x.T       32KB/part
            phiT_sb = big.tile([P, 2, HALF], F32R)  # phi_x.T         16KB/part
            gx_sb = big.tile([P, JT, C], F32R)      # g_x natural     32KB/part
            yT_sb = big.tile([P, 2, HALF], F32R)    # y.T             16KB/part
            resid_sb = big.tile([P, IT, C], F32)    # also output staging

            # ---- input DMAs (order = consumption order, alternate rings) ----
            def ld(i, dst, src):
                eng = nc.sync if i % 2 == 0 else nc.scalar
                eng.dma_start(out=dst, in_=src)

            xT_ap = xT_d.ap()
            ld(0, phW_sb, phW_d.ap())
            ld(1, xT_sb[:, :, 0:256], xT_ap[:, :, 0:256])
            ld(0, thW_sb, thW_d.ap())
            ld(1, xT_sb[:, :, 256:512], xT_ap[:, :, 256:512])
            ld(0, phb_sb, phb_d.ap())
            ld(1, thb_sb, thb_d.ap())
            ld(0, xT_sb[:, :, 512:1024], xT_ap[:, :, 512:1024])
            ld(1, xT_sb[:, :, 1024:1536], xT_ap[:, :, 1024:1536])
            ld(0, gW_sb, gW_d.ap())
            ld(1, gbb_sb, gbb_d.ap())
            for q in range(3, 8):
                ld(q, xT_sb[:, :, q * 512:(q + 1) * 512],
                   xT_ap[:, :, q * 512:(q + 1) * 512])
            ld(0, WW_sb, WW_d.ap())
            ld(1, resid_sb[:, :8, :], resid_d.ap()[:, :8, :])
            ld(0, resid_sb[:, 8:, :], resid_d.ap()[:, 8:, :])

            # ---- stage A, interleaved per 512-column chunk of xT ----
            # phi_x.T[d, i] (own rows = cols 0..2047), theta_x.T[d, j],
            # g_x[j, d] natural (g pre-scaled by 1/N; two j tiles per bank)
            for kc in range(8):
                # first chunk in 256-col halves so PE starts on less data
                subs = ([slice(0, 256), slice(256, 512)] if kc == 0
                        else [slice(kc * 512, (kc + 1) * 512)])
                if kc < 4:
                    for js in subs:
                        for dh in range(2):
                            ps = psw.tile([P, 512], F32, tag="work")
                            w = js.stop - js.start
                            for ch in range(2):
                                nc.tensor.matmul(
                                    ps[:, :w],
                                    phW_sb[:, ch, dh * P:(dh + 1) * P],
                                    xT_sb[:, ch, js],
                                    start=(ch == 0), stop=(ch == 1))
                            nc.scalar.activation(
                                out=phiT_sb[:, dh, js], in_=ps[:, :w],
                                func=AF.Identity,
                                bias=phb_sb[:, dh:dh + 1], scale=1.0)
                for js in subs:
                    for dh in range(2):
                        ps = psw.tile([P, 512], F32, tag="work")
                        w = js.stop - js.start
                        for ch in range(2):
                            nc.tensor.matmul(
                                ps[:, :w],
                                thW_sb[:, ch, dh * P:(dh + 1) * P],
                                xT_sb[:, ch, js],
                                start=(ch == 0), stop=(ch == 1))
                        nc.scalar.activation(
                            out=thetaT_sb[:, dh, js], in_=ps[:, :w],
                            func=AF.Identity,
                            bias=thb_sb[:, dh:dh + 1], scale=1.0)
                for q2 in range(2):
                    jp = kc * 2 + q2
                    pool, tg = (psw, "work") if jp % 2 == 0 else (psa, "acc")
                    ps = pool.tile([P, 512], F32, tag=tg)
                    for q in range(2):
                        jt = jp * 2 + q
                        for ch in range(2):
                            nc.tensor.matmul(
                                ps[:, q * C:(q + 1) * C],
                                xT_sb[:, ch, jt * P:(jt + 1) * P],
                                gW_sb[:, ch, :],
                                start=(ch == 0), stop=(ch == 1))
                    nc.vector.tensor_add(
                        out=gx_sb[:, jp * 2:jp * 2 + 2, :],
                        in0=ps.rearrange("p (t d) -> p t d", d=C),
                        in1=gbb_sb)

            # ---- stages B+C, software-pipelined over j tiles ----
            # B: fT[j, i] = sum_d thetaT[d, j] * phiT[d, i]
            # C: yT[d', i] += sum_j gx[j, d'] * fT[j, i]
            for ih in range(2):
                psC = [psa.tile([P, 512], F32, tag="acc", name=f"psC{q}")
                       for q in range(4)]
                fTs = {}

                def stage_b(jt, ih=ih, fTs=fTs):
                    fT = fTp.tile([P, 2, 512], F32R, tag="fT", name=f"fT{jt}")
                    fTs[jt] = fT
                    for ck in range(2):
                        ps = psw.tile([P, 512], F32, tag="work",
                                      name=f"psB{jt}_{ck}")
                        isl = slice(ih * 1024 + ck * 512,
                                    ih * 1024 + (ck + 1) * 512)
                        for dh in range(2):
                            nc.tensor.matmul(
                                ps,
                                thetaT_sb[:, dh, jt * P:(jt + 1) * P],
                                phiT_sb[:, dh, isl],
                                start=(dh == 0), stop=(dh == 1))
                        if ck == 0:
                            nc.vector.tensor_copy(out=fT[:, ck, :], in_=ps)
                        else:
                            nc.scalar.copy(out=fT[:, ck, :], in_=ps)

                def stage_c(jt, psC=psC, fTs=fTs):
                    fT = fTs.pop(jt)
                    for dp in range(2):
                        for ck in range(2):
                            nc.tensor.matmul(
                                psC[dp * 2 + ck],
                                gx_sb[:, jt, dp * P:(dp + 1) * P],
                                fT[:, ck, :],
                                start=(jt == 0), stop=(jt == JT - 1))

                stage_b(0)
                for jt in range(1, JT):
                    stage_b(jt)
                    stage_c(jt - 1)
                stage_c(JT - 1)

                for dp in range(2):
                    for ck in range(2):
                        isl = slice(ih * 1024 + ck * 512,
                                    ih * 1024 + (ck + 1) * 512)
                        if ck == 0:
                            nc.vector.tensor_copy(out=yT_sb[:, dp, isl],
                                                  in_=psC[dp * 2 + ck])
                        else:
                            nc.scalar.copy(out=yT_sb[:, dp, isl],
                                           in_=psC[dp * 2 + ck])

                # ---- stage D for this half: out = yT.T @ WW + resid ----
                # two i tiles per PSUM bank; one DVE add + out DMA per pair
                for itp in range(4):
                    it0 = ih * 8 + itp * 2
                    ps = psw.tile([P, 512], F32, tag="work")
                    for q in range(2):
                        it = it0 + q
                        for dp in range(2):
                            nc.tensor.matmul(
                                ps[:, q * C:(q + 1) * C],
                                yT_sb[:, dp, it * P:(it + 1) * P],
                                WW_sb[:, dp, :],
                                start=(dp == 0), stop=(dp == 1))
                    nc.vector.tensor_add(
                        out=resid_sb[:, it0:it0 + 2, :],
                        in0=ps.rearrange("p (t d) -> p t d", d=C),
                        in1=resid_sb[:, it0:it0 + 2, :])
                    eng = nc.sync if itp % 2 == 0 else nc.scalar
                    eng.dma_start(out=out_d.ap()[:, it0:it0 + 2, :],
                                  in_=resid_sb[:, it0:it0 + 2, :])

    nc.finalize()
    return nc


def _get_module():
    if "nc" not in _CACHE:
        _CACHE["nc"] = _build_module()
    return _CACHE["nc"]


def _to_sbuf_layout(a):
    """[(o*128+p), F...] -> [128, o, F...] contiguous."""
    o = a.shape[0] // P
    return np.ascontiguousarray(
        a.reshape(o, P, *a.shape[1:]).swapaxes(0, 1))


def _prep_in_maps(x, g_w, g_b, theta_w, theta_b, phi_w, phi_b, W_w, W_b):
    x = np.ascontiguousarray(np.asarray(x, dtype=np.float32))
    f32 = np.float32

    def col2(v):  # [256] -> [128, 2] (column h = channels h*128..h*128+127)
        return np.ascontiguousarray(np.asarray(v, f32).reshape(2, P).T)

    thW = _to_sbuf_layout(np.ascontiguousarray(np.asarray(theta_w, f32).T))
    phW = _to_sbuf_layout(np.ascontiguousarray(np.asarray(phi_w, f32).T))
    gW = _to_sbuf_layout(np.ascontiguousarray(np.asarray(g_w, f32).T / N))
    WW = _to_sbuf_layout(np.ascontiguousarray(np.asarray(W_w, f32).T))
    thb = col2(theta_b)
    phb = col2(phi_b)
    gbb = np.ascontiguousarray(np.broadcast_to(
        np.asarray(g_b, f32) / N, (P, 2, C)))
    W_b = np.asarray(W_b, f32)

    in_maps = []
    for core in range(NCORES):
        b, h = core // 2, core % 2
        rows = slice(h * HALF, (h + 1) * HALF)
        other = slice(0, HALF) if h else slice(HALF, N)
        xb = x[b]
        xrot_T = np.concatenate([xb[rows], xb[other]], axis=0).T  # [C, N]
        resid = xb[rows] + W_b                                     # [HALF, C]
        in_maps.append({
            "xT": _to_sbuf_layout(np.ascontiguousarray(xrot_T)),
            "thW": thW, "phW": phW, "gW": gW, "WW": WW,
            "thb": thb, "phb": phb, "gbb": gbb,
            "resid": _to_sbuf_layout(resid),
        })
    return in_maps


def kernel(x, g_w, g_b, theta_w, theta_b, phi_w, phi_b, W_w, W_b):
    nc = _get_module()
    in_maps = _prep_in_maps(x, g_w, g_b, theta_w, theta_b, phi_w, phi_b,
                            W_w, W_b)
    res = run_bass_kernel_spmd(nc, in_maps, core_ids=list(range(NCORES)))
    out = np.empty((B, N, C), dtype=np.float32)
    for core in range(NCORES):
        b, h = core // 2, core % 2
        o = res.results[core]["out"]  # [128, 16, 256] = [p, t, d]
        out[b, h * HALF:(h + 1) * HALF, :] = (
            o.swapaxes(0, 1).reshape(HALF, C))
    return out
